# revision 12
# baseline (speedup 1.0000x reference)
"""ConvAttention (linear attention with conv projections) on 8 trn2 cores.

Sharding: data-parallel over batch B=8, one image per NeuronCore.

v2 pipeline (channel-major activations [chan, tok], tok = y*64+x):
  qproj   PE -> exp (ACT, [128,1024] tiles) -> expq sbuf
  dw      depthwise3x3: ct0 on GPSIMD (9 STT), ct1 on DVE (TS 4x + TT-add 2x),
          chunked by 16 y-rows to overlap with the kv chain
  kv      PE token-major psum [128,1024]; exp_k (ACT) -> ring; v copy -> ring
  ctx     PE [128,129] blocks: v augmented with ones column so col 128
          accumulates Sk per partition (no separate row-sum / transpose)
  ctxn    DVE scale by 1/Sk * scale into block-diag tile
  Sq      PE bdiag matmul -> psum; 1/Sq on ACT Reciprocal LUT (batched after
          all exps: one table swap) -> rb sbuf
  eqn     DVE expq * rb
  att     PE ctxn^T @ eqn
  gelu    ACT psum -> bf16 sbuf (reuses expq storage)
  out     PE Wout @ g + bias via K=1 ones matmul; DMA psum -> DRAM f32
"""

import numpy as np
import ml_dtypes

B, C, H, W = 8, 256, 64, 64
HEADS, HID = 8, 64
TMP = HEADS * HID            # 512
N = H * W                    # 4096
PAD = 66                     # 64 + 2 halo
NPAD = PAD * PAD             # 4356
NT = 32                      # token tiles of 128
NCHUNK = 4                   # dw/kv chunks of 16 y-rows (1024 tokens)
RING = 4                     # expk / vsb ring depth (token tiles)
SCALE = float(HID) ** -0.5

_CACHE = {}


def _build(debug=False):
    from contextlib import ExitStack

    import concourse.bass as bass
    import concourse.mybir as mybir
    import concourse.tile as tile
    from concourse import bacc

    dt = mybir.dt
    f32, bf16 = dt.float32, dt.bfloat16
    Al = mybir.AluOpType
    Act = mybir.ActivationFunctionType

    nc = bacc.Bacc(
        "TRN2", target_bir_lowering=False, debug=False, enable_asserts=False
    )

    din = {}
    for name, shape, d in [
        ("fpa", [128, 2, NPAD], bf16),       # pad(1,1): x data at cols 1..64
        ("fpb", [128, 2, NPAD], bf16),       # pad(2,0): x data at cols 2..65
        ("wq", [128, 2, TMP], bf16),         # Wq^T   [c, o]
        ("wkv", [128, 2, 2 * TMP], bf16),    # Wkv^T  [c, o]
        ("wout", [128, 4, C], bf16),         # Wout^T [o, c]
        ("wdw", [128, 2, 9], f32),           # depthwise taps per channel
        ("wdiag", [128, 9, 128], bf16),      # diag(tap) matrices, ct0 chans
        ("bout2", [128, 2], f32),            # bias, c-tiled columns
        ("bdiag", [128, 128], bf16),         # [[J,0],[0,J]] 64x64 ones blocks
    ]:
        din[name] = nc.dram_tensor(name, shape, d, kind="ExternalInput").ap()
    out_d = nc.dram_tensor("out", [2, 128, N], f32, kind="ExternalOutput").ap()
    dbg = {}
    if debug:
        for name, shape, d in [
            ("d_dw", [128, 2, N], bf16),
            ("d_expq", [128, 4, N], bf16),
            ("d_rsk", [128, 4], f32),
            ("d_ctxn", [128, 4, 128], bf16),
            ("d_rb", [128, 4, N], bf16),
        ]:
            dbg[name] = nc.dram_tensor(
                name, shape, d, kind="ExternalOutput").ap()

    with tile.TileContext(nc) as tc, ExitStack() as ctx:
        wp = ctx.enter_context(tc.tile_pool(name="wp", bufs=1))
        sb = ctx.enter_context(tc.tile_pool(name="sb", bufs=1))

        # ---- constants / weights -------------------------------------------
        wq = wp.tile([128, 2, TMP], bf16)
        wkv = wp.tile([128, 2, 2 * TMP], bf16)
        wout = wp.tile([128, 4, C], bf16)
        wdw = wp.tile([128, 2, 9], f32)
        wdiag = wp.tile([128, 9, 128], bf16)
        bout2 = wp.tile([128, 2], f32)
        bdiag = wp.tile([128, 128], bf16)
        # input images in 4 row-bands so early chunks start sooner; the
        # first band and the weights feeding the first matmuls go first
        fpa = sb.tile([128, 2, NPAD], bf16)
        fpb = sb.tile([128, 2, NPAD], bf16)
        bands = [(0, 18), (18, 34), (34, 50), (50, 66)]

        def band_dma(r0, r1):
            sl = slice(r0 * PAD, r1 * PAD)
            nc.sync.dma_start(out=fpa[:, :, sl], in_=din["fpa"][:, :, sl])
            nc.sync.dma_start(out=fpb[:, :, sl], in_=din["fpb"][:, :, sl])

        band_dma(*bands[0])
        for t, name in [(wq, "wq"), (wdw, "wdw"), (wdiag, "wdiag")]:
            nc.sync.dma_start(out=t, in_=din[name])
        band_dma(*bands[1])
        for t, name in [(wkv, "wkv"), (wout, "wout"),
                        (bout2, "bout2"), (bdiag, "bdiag")]:
            nc.sync.dma_start(out=t, in_=din[name])
        for b in bands[2:]:
            band_dma(*b)

        # ---- big sbuf tensors ----------------------------------------------
        dw = sb.tile([128, 2, N], bf16)       # depthwise output, channel-major
        tmpv = sb.tile([128, 1024], bf16)     # DVE tap staging
        dwx = sb.tile([128, 1024], bf16)      # DVE share of ct0 taps
        expq = sb.tile([128, 4, N], bf16)     # exp(q); later reused as g
        expk = sb.tile([128, RING, 512], bf16)   # token-major ring
        vsb = sb.tile([128, RING, 4, 132], bf16)  # v ring + ones col at 128
        ctxn = sb.tile([128, 4, 128], bf16)   # block-diag scaled ctx per pair
        rsk = sb.tile([128, 4], f32)
        rb = sb.tile([128, 4, N], bf16)       # 1/Sq broadcast per head pair

        nc.gpsimd.memset(vsb[:, :, :, 128:129], 1.0)
        nc.gpsimd.memset(ctxn, 0.0)

        def fview(ct, dy, dx, y0, ny):
            # padded image view [128, ny, 64] for tap (dy, dx), rows y0..y0+ny
            x0 = 1 + dx if dx != 0 else 2
            src = fpa if dx != 0 else fpb
            im = src[:, ct].rearrange("p (y x) -> p y x", y=PAD)
            return im[:, 1 + dy + y0:1 + dy + y0 + ny, x0:x0 + 64]

        def qview(ct, y0, ny):
            im = fpa[:, ct].rearrange("p (y x) -> p y x", y=PAD)
            return im[:, 1 + y0:1 + y0 + ny, 1:65]

        ctxA = ctx.enter_context(ExitStack())
        pa = ctxA.enter_context(
            tc.tile_pool(name="pa", bufs=2, space="PSUM"))
        phC = ctxA.enter_context(
            tc.tile_pool(name="phC", bufs=2, space="PSUM"))
        ctxt = [phC.tile([128, 2, 129], f32, tag="ctx", name=f"ctxt{i}")
                for i in range(2)]

        taps = [(dy, dx) for dy in (-1, 0, 1) for dx in (-1, 0, 1)]

        # ---- PE warmup during DMA lead-in (HAM un-throttle) ----------------
        wps = pa.tile([128, 1024], f32, tag="pa")
        for i in range(16):
            nc.tensor.matmul(
                wps[:, 0:128], wq[:, 0, 0:128], wq[:, 1, 0:128],
                start=(i == 0), stop=(i == 15), skip_group_check=True)

        # ---- q projection + exp (channel-major) ----------------------------
        def emit_qp(ot):
            osl = slice(ot * 128, (ot + 1) * 128)
            for ch in range(NCHUNK):
                ps = pa.tile([128, 1024], f32, tag="pa")
                for hf in range(2):
                    y0 = ch * 16 + hf * 8
                    for ct in range(2):
                        nc.tensor.matmul(
                            ps[:, hf * 512:(hf + 1) * 512],
                            wq[:, ct, osl], qview(ct, y0, 8),
                            start=(ct == 0), stop=(ct == 1))
                nc.scalar.activation(
                    expq[:, ot, ch * 1024:(ch + 1) * 1024], ps, Act.Exp)

        # ---- depthwise + kv + ctx, pipelined chunk emission ----------------
        def emit_dw(ch):
            y0 = ch * 16
            csl = slice(ch * 1024, (ch + 1) * 1024)
            # ct0: taps 0-6 on PE diag matmuls, taps 7-8 + merge on DVE
            dwp = pa.tile([128, 1024], f32, tag="pa")
            for i, (dy, dx) in enumerate(taps[:7]):
                for hf in range(2):
                    nc.tensor.matmul(
                        dwp[:, hf * 512:(hf + 1) * 512], wdiag[:, i],
                        fview(0, dy, dx, y0 + hf * 8, 8),
                        start=(i == 0), stop=(i == 6))
            nc.scalar.copy(dw[:, 0, csl], dwp)
            # ct1 on DVE: tensor_scalar 4x + tensor_tensor add 2x
            dwv = dw[:, 1, csl].rearrange("p (y x) -> p y x", y=16)
            tmp3 = tmpv.rearrange("p (y x) -> p y x", y=16)
            for i, (dy, dx) in enumerate(taps):
                fv = fview(1, dy, dx, y0, 16)
                if i == 0:
                    nc.vector.tensor_scalar_mul(dwv, fv, wdw[:, 1, 0:1])
                else:
                    nc.vector.tensor_scalar_mul(tmp3, fv, wdw[:, 1, i:i + 1])
                    nc.vector.tensor_add(dwv, dwv, tmp3)
            # DVE share of ct0: taps 7,8 into dwx, then merged onto dw ct0
            dwx3 = dwx.rearrange("p (y x) -> p y x", y=16)
            for i, (dy, dx) in list(enumerate(taps))[7:]:
                fv = fview(0, dy, dx, y0, 16)
                if i == 7:
                    nc.vector.tensor_scalar_mul(dwx3, fv, wdw[:, 0, 7:8])
                else:
                    nc.vector.tensor_scalar_mul(tmp3, fv, wdw[:, 0, i:i + 1])
                    nc.vector.tensor_add(dwx3, dwx3, tmp3)
            nc.vector.tensor_add(dw[:, 0, csl], dw[:, 0, csl], dwx)

        def emit_kv(ch):
            for tt in range(ch * 8, ch * 8 + 8):
                tsl = slice(tt * 128, (tt + 1) * 128)
                r = tt % RING
                ps = pa.tile([128, 1024], f32, tag="pa")
                for ct in range(2):
                    nc.tensor.matmul(
                        ps[:, 0:512], dw[:, ct, tsl], wkv[:, ct, 0:512],
                        start=(ct == 0), stop=(ct == 1))
                    nc.tensor.matmul(
                        ps[:, 512:1024], dw[:, ct, tsl], wkv[:, ct, 512:1024],
                        start=(ct == 0), stop=(ct == 1))
                nc.scalar.activation(expk[:, r], ps[:, 0:512], Act.Exp)
                vdst = vsb[:, r, :, 0:128]
                vsrc = ps[:, 512:1024].rearrange("p (a b) -> p a b", a=4)
                nc.scalar.copy(vdst, vsrc)
                for pr in range(4):
                    psl = slice(pr * 128, (pr + 1) * 128)
                    # start=True zeroes the whole 2KB psum bank; only the
                    # first region per bank may use it (pr%2==1 accumulates
                    # onto the bank just zeroed by its pr%2==0 sibling).
                    nc.tensor.matmul(
                        ctxt[pr // 2][:, pr % 2], expk[:, r, psl],
                        vsb[:, r, pr, 0:129],
                        start=(tt == 0 and pr % 2 == 0),
                        stop=(tt == NT - 1),
                        skip_group_check=True)

        phS = ctxA.enter_context(
            tc.tile_pool(name="phS", bufs=2, space="PSUM"))
        rbp32 = ctx.enter_context(tc.tile_pool(name="rbp32", bufs=2))

        def emit_b1(ot):
            # Sq via bdiag matmul; 1/Sq approx + bf16 cast on DVE
            rb32 = rbp32.tile([128, N], f32, tag="rb32")
            for j in range(8):
                base = j * 512
                sqt = phS.tile([128, 512], f32, tag="sq")
                nc.tensor.matmul(
                    sqt, bdiag, expq[:, ot, base:base + 512],
                    start=True, stop=True)
                nc.vector.reciprocal_approx_fast(
                    out=rb32[:, base:base + 512], in_=sqt)
                nc.vector.tensor_copy(
                    rb[:, ot, base:base + 512], rb32[:, base:base + 512])

        emit_dw(0)
        emit_dw(1)
        emit_kv(0)
        emit_qp(0)
        emit_b1(0)
        emit_dw(2)
        emit_kv(1)
        emit_qp(1)
        emit_b1(1)
        emit_dw(3)
        emit_kv(2)
        emit_qp(2)
        emit_b1(2)
        emit_kv(3)
        emit_qp(3)
        emit_b1(3)

        # ---- Sk reciprocal + ctxn block-diag build -------------------------
        for pr in range(4):
            nc.vector.reciprocal(
                rsk[:, pr:pr + 1], ctxt[pr // 2][:, pr % 2, 128:129])
        for pr in range(4):
            for hh in range(2):
                rs = slice(hh * 64, (hh + 1) * 64)
                nc.vector.tensor_scalar(
                    out=ctxn[rs, pr, hh * 64:hh * 64 + 64],
                    in0=ctxt[pr // 2][rs, pr % 2, hh * 64:hh * 64 + 64],
                    scalar1=rsk[rs, pr:pr + 1], scalar2=SCALE,
                    op0=Al.mult, op1=Al.mult)
        if debug:
            nc.sync.dma_start(out=dbg["d_dw"], in_=dw)
            nc.sync.dma_start(out=dbg["d_expq"], in_=expq)
            nc.sync.dma_start(out=dbg["d_rsk"], in_=rsk)
            nc.sync.dma_start(out=dbg["d_ctxn"], in_=ctxn)

        ctxA.close()  # release pa + phC + phS
        if debug:
            nc.sync.dma_start(out=dbg["d_rb"], in_=rb)

        # ---- phase B2: eqn, att, gelu, out projection ----------------------
        eqp = ctx.enter_context(tc.tile_pool(name="eqp", bufs=12))
        attp = ctx.enter_context(
            tc.tile_pool(name="attp", bufs=3, space="PSUM"))
        outp = ctx.enter_context(
            tc.tile_pool(name="outp", bufs=2, space="PSUM"))
        osb = ctx.enter_context(tc.tile_pool(name="osb", bufs=3))

        def emit_att(c):
            csl = slice(c * 512, (c + 1) * 512)
            for ot in range(4):
                eqn = eqp.tile([128, 512], bf16, tag="eqn")
                nc.vector.tensor_mul(eqn, expq[:, ot, csl], rb[:, ot, csl])
                att = attp.tile([128, 512], f32, tag="att")
                nc.tensor.matmul(att, ctxn[:, ot], eqn, start=True, stop=True)
                nc.scalar.activation(expq[:, ot, csl], att, Act.Gelu)

        def emit_out(c):
            csl = slice(c * 512, (c + 1) * 512)
            for ct in range(2):
                op = outp.tile([128, 512], f32, tag="op")
                for ot in range(4):
                    nc.tensor.matmul(
                        op, wout[:, ot, ct * 128:(ct + 1) * 128],
                        expq[:, ot, csl], start=(ot == 0), stop=(ot == 3))
                ot_sb = osb.tile([128, 512], f32, tag="osb")
                if ct == 0:
                    nc.vector.tensor_scalar_add(ot_sb, op, bout2[:, ct:ct + 1])
                else:
                    nc.scalar.activation(
                        ot_sb, op, Act.Identity, bias=bout2[:, ct:ct + 1])
                nc.sync.dma_start(out=out_d[ct, :, csl], in_=ot_sb)

        emit_att(0)
        for c in range(1, 8):
            emit_att(c)
            emit_out(c - 1)
        emit_out(7)

    nc.compile()
    return nc


def _prep_inputs(fmap, Wq, Wdw, Wkv, Wout, bout):
    bf16 = ml_dtypes.bfloat16
    f32 = np.float32

    def ctile(a):  # [256, X] -> [128, 2, X]
        return np.ascontiguousarray(
            a.reshape(2, 128, *a.shape[1:]).transpose(1, 0, *range(2, a.ndim + 1)))

    shared = {
        "wq": ctile(Wq.T.astype(bf16)),
        "wkv": ctile(Wkv.T.astype(bf16)),
        "wout": np.ascontiguousarray(
            Wout.T.astype(bf16).reshape(4, 128, C).transpose(1, 0, 2)),
        "wdw": np.ascontiguousarray(
            Wdw.reshape(C, 9).reshape(2, 128, 9).transpose(1, 0, 2).astype(f32)),
        "wdiag": np.ascontiguousarray(
            np.stack([np.diag(Wdw.reshape(C, 9)[:128, i]) for i in range(9)],
                     axis=1).astype(bf16)),
        "bout2": np.ascontiguousarray(bout.astype(f32).reshape(2, 128).T),
        "bdiag": np.kron(np.eye(2, dtype=f32), np.ones((64, 64), f32)).astype(bf16),
    }
    in_maps = []
    for b in range(B):
        fpa = np.pad(fmap[b], [(0, 0), (1, 1), (1, 1)]).astype(bf16)
        fpb = np.pad(fmap[b], [(0, 0), (1, 1), (2, 0)]).astype(bf16)
        m = dict(shared)
        m["fpa"] = ctile(fpa.reshape(C, NPAD))
        m["fpb"] = ctile(fpb.reshape(C, NPAD))
        in_maps.append(m)
    return in_maps


def kernel(fmap, Wq, Wdw, Wkv, Wout, bout, _trace=False, _tmpdir=None,
           _debug=False):
    from concourse.bass_utils import run_bass_kernel_spmd

    fmap, Wq, Wdw, Wkv, Wout, bout = (
        np.asarray(a, np.float32) for a in (fmap, Wq, Wdw, Wkv, Wout, bout))

    key = "nc_dbg" if _debug else "nc"
    if key not in _CACHE:
        _CACHE[key] = _build(debug=_debug)
    nc = _CACHE[key]

    in_maps = _prep_inputs(fmap, Wq, Wdw, Wkv, Wout, bout)
    res = run_bass_kernel_spmd(
        nc, in_maps, core_ids=list(range(B)), trace=_trace, tmpdir=_tmpdir)
    _CACHE["last_result"] = res
    out = np.stack([r["out"] for r in res.results])        # [B, 2, 128, N]
    return out.reshape(B, C, H, W).astype(np.float32)


# revision 13
# speedup vs baseline: 1.0037x; 1.0037x over previous
"""ConvAttention (linear attention with conv projections) on 8 trn2 cores.

Sharding: data-parallel over batch B=8, one image per NeuronCore.

Pipeline (channel-major activations [chan, tok], tok = y*64+x):
  warmup  16 small matmuls during DMA lead-in un-throttle the PE clock (HAM)
  dw      depthwise3x3 split: ct0 on PE (9 diag-matmul taps, f32 psum
          accumulate, ACT copy out), ct1 on DVE (tensor_scalar 4x +
          tensor_tensor add 2x), chunked by 16 y-rows
  qproj   PE -> exp (ACT, [128,1024] psum tiles) -> expq bf16 sbuf
  kv      PE token-major psum [128,1024] per 128-token tile; exp_k (ACT)
          and v copy (ACT) into 4-deep rings
  ctx     PE [128,129] blocks: v ring carries a ones column at col 128 so
          the ctx matmul also accumulates Sk per partition (no transpose);
          two accumulation regions share a psum bank, so only the first
          uses start=True (start zeroes the whole 2KB zero region)
  Sq      PE bdiag matmul -> 1/Sq via DVE reciprocal_approx_fast (f32) ->
          bf16 cast, all overlapped into phase A between kv chunks
  eqn     DVE expq * rb, precomputed into a 12-deep pool
  att     PE ctxn^T @ eqn; gelu (ACT) overwrites expq storage
  out     PE Wout @ g; bias + f32 copy fused on DVE tensor_scalar_add;
          DMA sbuf -> DRAM f32
"""

import numpy as np
import ml_dtypes

B, C, H, W = 8, 256, 64, 64
HEADS, HID = 8, 64
TMP = HEADS * HID            # 512
N = H * W                    # 4096
PAD = 66                     # 64 + 2 halo
NPAD = PAD * PAD             # 4356
NT = 32                      # token tiles of 128
NCHUNK = 4                   # dw/kv chunks of 16 y-rows (1024 tokens)
RING = 4                     # expk / vsb ring depth (token tiles)
SCALE = float(HID) ** -0.5

_CACHE = {}


def _build(debug=False):
    from contextlib import ExitStack

    import concourse.bass as bass
    import concourse.mybir as mybir
    import concourse.tile as tile
    from concourse import bacc

    dt = mybir.dt
    f32, bf16 = dt.float32, dt.bfloat16
    Al = mybir.AluOpType
    Act = mybir.ActivationFunctionType

    nc = bacc.Bacc(
        "TRN2", target_bir_lowering=False, debug=False, enable_asserts=False
    )

    din = {}
    for name, shape, d in [
        ("fpa", [128, 2, NPAD], bf16),       # pad(1,1): x data at cols 1..64
        ("fpb", [128, 2, NPAD], bf16),       # pad(2,0): x data at cols 2..65
        ("wq", [128, 2, TMP], bf16),         # Wq^T   [c, o]
        ("wkv", [128, 2, 2 * TMP], bf16),    # Wkv^T  [c, o]
        ("wout", [128, 4, C], bf16),         # Wout^T [o, c]
        ("wdw", [128, 2, 9], f32),           # depthwise taps per channel
        ("wdiag", [128, 9, 128], bf16),      # diag(tap) matrices, ct0 chans
        ("bout2", [128, 2], f32),            # bias, c-tiled columns
        ("bdiag", [128, 128], bf16),         # [[J,0],[0,J]] 64x64 ones blocks
    ]:
        din[name] = nc.dram_tensor(name, shape, d, kind="ExternalInput").ap()
    out_d = nc.dram_tensor("out", [2, 128, N], f32, kind="ExternalOutput").ap()
    dbg = {}
    if debug:
        for name, shape, d in [
            ("d_dw", [128, 2, N], bf16),
            ("d_expq", [128, 4, N], bf16),
            ("d_rsk", [128, 4], f32),
            ("d_ctxn", [128, 4, 128], bf16),
            ("d_rb", [128, 4, N], bf16),
        ]:
            dbg[name] = nc.dram_tensor(
                name, shape, d, kind="ExternalOutput").ap()

    with tile.TileContext(nc) as tc, ExitStack() as ctx:
        wp = ctx.enter_context(tc.tile_pool(name="wp", bufs=1))
        sb = ctx.enter_context(tc.tile_pool(name="sb", bufs=1))

        # ---- constants / weights -------------------------------------------
        wq = wp.tile([128, 2, TMP], bf16)
        wkv = wp.tile([128, 2, 2 * TMP], bf16)
        wout = wp.tile([128, 4, C], bf16)
        wdw = wp.tile([128, 2, 9], f32)
        wdiag = wp.tile([128, 9, 128], bf16)
        bout2 = wp.tile([128, 2], f32)
        bdiag = wp.tile([128, 128], bf16)
        # input images in 4 row-bands so early chunks start sooner; the
        # first band and the weights feeding the first matmuls go first
        fpa = sb.tile([128, 2, NPAD], bf16)
        fpb = sb.tile([128, 2, NPAD], bf16)
        bands = [(0, 18), (18, 34), (34, 50), (50, 66)]

        def band_dma(r0, r1):
            sl = slice(r0 * PAD, r1 * PAD)
            nc.sync.dma_start(out=fpa[:, :, sl], in_=din["fpa"][:, :, sl])
            nc.sync.dma_start(out=fpb[:, :, sl], in_=din["fpb"][:, :, sl])

        band_dma(*bands[0])
        for t, name in [(wq, "wq"), (wdw, "wdw"), (wdiag, "wdiag")]:
            nc.sync.dma_start(out=t, in_=din[name])
        band_dma(*bands[1])
        for t, name in [(wkv, "wkv"), (wout, "wout"),
                        (bout2, "bout2"), (bdiag, "bdiag")]:
            nc.sync.dma_start(out=t, in_=din[name])
        for b in bands[2:]:
            band_dma(*b)

        # ---- big sbuf tensors ----------------------------------------------
        dw = sb.tile([128, 2, N], bf16)       # depthwise output, channel-major
        tmpv = sb.tile([128, 1024], bf16)     # DVE tap staging
        expq = sb.tile([128, 4, N], bf16)     # exp(q); later reused as g
        expk = sb.tile([128, RING, 512], bf16)   # token-major ring
        vsb = sb.tile([128, RING, 4, 132], bf16)  # v ring + ones col at 128
        ctxn = sb.tile([128, 4, 128], bf16)   # block-diag scaled ctx per pair
        rsk = sb.tile([128, 4], f32)
        rb = sb.tile([128, 4, N], bf16)       # 1/Sq broadcast per head pair

        nc.gpsimd.memset(vsb[:, :, :, 128:129], 1.0)
        nc.gpsimd.memset(ctxn, 0.0)

        def fview(ct, dy, dx, y0, ny):
            # padded image view [128, ny, 64] for tap (dy, dx), rows y0..y0+ny
            x0 = 1 + dx if dx != 0 else 2
            src = fpa if dx != 0 else fpb
            im = src[:, ct].rearrange("p (y x) -> p y x", y=PAD)
            return im[:, 1 + dy + y0:1 + dy + y0 + ny, x0:x0 + 64]

        def qview(ct, y0, ny):
            im = fpa[:, ct].rearrange("p (y x) -> p y x", y=PAD)
            return im[:, 1 + y0:1 + y0 + ny, 1:65]

        ctxA = ctx.enter_context(ExitStack())
        pa = ctxA.enter_context(
            tc.tile_pool(name="pa", bufs=2, space="PSUM"))
        phC = ctxA.enter_context(
            tc.tile_pool(name="phC", bufs=2, space="PSUM"))
        ctxt = [phC.tile([128, 2, 129], f32, tag="ctx", name=f"ctxt{i}")
                for i in range(2)]

        taps = [(dy, dx) for dy in (-1, 0, 1) for dx in (-1, 0, 1)]

        # ---- PE warmup during DMA lead-in (HAM un-throttle) ----------------
        wps = pa.tile([128, 1024], f32, tag="pa")
        for i in range(16):
            nc.tensor.matmul(
                wps[:, 0:128], wq[:, 0, 0:128], wq[:, 1, 0:128],
                start=(i == 0), stop=(i == 15), skip_group_check=True)

        # ---- q projection + exp (channel-major) ----------------------------
        def emit_qp(ot):
            osl = slice(ot * 128, (ot + 1) * 128)
            for ch in range(NCHUNK):
                ps = pa.tile([128, 1024], f32, tag="pa")
                for hf in range(2):
                    y0 = ch * 16 + hf * 8
                    for ct in range(2):
                        nc.tensor.matmul(
                            ps[:, hf * 512:(hf + 1) * 512],
                            wq[:, ct, osl], qview(ct, y0, 8),
                            start=(ct == 0), stop=(ct == 1))
                nc.scalar.activation(
                    expq[:, ot, ch * 1024:(ch + 1) * 1024], ps, Act.Exp)

        # ---- depthwise + kv + ctx, pipelined chunk emission ----------------
        def emit_dw(ch):
            y0 = ch * 16
            csl = slice(ch * 1024, (ch + 1) * 1024)
            # ct0 on PE: 9 diag-matmul taps accumulated in psum, ACT copy out
            dwp = pa.tile([128, 1024], f32, tag="pa")
            for i, (dy, dx) in enumerate(taps):
                for hf in range(2):
                    nc.tensor.matmul(
                        dwp[:, hf * 512:(hf + 1) * 512], wdiag[:, i],
                        fview(0, dy, dx, y0 + hf * 8, 8),
                        start=(i == 0), stop=(i == 8))
            nc.scalar.copy(dw[:, 0, csl], dwp)
            # ct1 on DVE: tensor_scalar 4x + tensor_tensor add 2x
            dwv = dw[:, 1, csl].rearrange("p (y x) -> p y x", y=16)
            tmp3 = tmpv.rearrange("p (y x) -> p y x", y=16)
            for i, (dy, dx) in enumerate(taps):
                fv = fview(1, dy, dx, y0, 16)
                if i == 0:
                    nc.vector.tensor_scalar_mul(dwv, fv, wdw[:, 1, 0:1])
                else:
                    nc.vector.tensor_scalar_mul(tmp3, fv, wdw[:, 1, i:i + 1])
                    nc.vector.tensor_add(dwv, dwv, tmp3)

        def emit_kv(ch):
            for tt in range(ch * 8, ch * 8 + 8):
                tsl = slice(tt * 128, (tt + 1) * 128)
                r = tt % RING
                ps = pa.tile([128, 1024], f32, tag="pa")
                for ct in range(2):
                    nc.tensor.matmul(
                        ps[:, 0:512], dw[:, ct, tsl], wkv[:, ct, 0:512],
                        start=(ct == 0), stop=(ct == 1))
                    nc.tensor.matmul(
                        ps[:, 512:1024], dw[:, ct, tsl], wkv[:, ct, 512:1024],
                        start=(ct == 0), stop=(ct == 1))
                nc.scalar.activation(expk[:, r], ps[:, 0:512], Act.Exp)
                vdst = vsb[:, r, :, 0:128]
                vsrc = ps[:, 512:1024].rearrange("p (a b) -> p a b", a=4)
                nc.scalar.copy(vdst, vsrc)
                for pr in range(4):
                    psl = slice(pr * 128, (pr + 1) * 128)
                    # start=True zeroes the whole 2KB psum bank; only the
                    # first region per bank may use it (pr%2==1 accumulates
                    # onto the bank just zeroed by its pr%2==0 sibling).
                    nc.tensor.matmul(
                        ctxt[pr // 2][:, pr % 2], expk[:, r, psl],
                        vsb[:, r, pr, 0:129],
                        start=(tt == 0 and pr % 2 == 0),
                        stop=(tt == NT - 1),
                        skip_group_check=True)

        phS = ctxA.enter_context(
            tc.tile_pool(name="phS", bufs=2, space="PSUM"))
        rbp32 = ctx.enter_context(tc.tile_pool(name="rbp32", bufs=2))

        def emit_b1(ot):
            # Sq via bdiag matmul; 1/Sq approx + bf16 cast on DVE
            rb32 = rbp32.tile([128, N], f32, tag="rb32")
            for j in range(8):
                base = j * 512
                sqt = phS.tile([128, 512], f32, tag="sq")
                nc.tensor.matmul(
                    sqt, bdiag, expq[:, ot, base:base + 512],
                    start=True, stop=True)
                nc.vector.reciprocal_approx_fast(
                    out=rb32[:, base:base + 512], in_=sqt)
                nc.vector.tensor_copy(
                    rb[:, ot, base:base + 512], rb32[:, base:base + 512])

        emit_dw(0)
        emit_dw(1)
        emit_kv(0)
        emit_qp(0)
        emit_b1(0)
        emit_dw(2)
        emit_kv(1)
        emit_qp(1)
        emit_b1(1)
        emit_dw(3)
        emit_kv(2)
        emit_qp(2)
        emit_b1(2)
        emit_kv(3)
        emit_qp(3)
        emit_b1(3)

        # ---- Sk reciprocal + ctxn block-diag build -------------------------
        for pr in range(4):
            nc.vector.reciprocal(
                rsk[:, pr:pr + 1], ctxt[pr // 2][:, pr % 2, 128:129])
        for pr in range(4):
            for hh in range(2):
                rs = slice(hh * 64, (hh + 1) * 64)
                nc.vector.tensor_scalar(
                    out=ctxn[rs, pr, hh * 64:hh * 64 + 64],
                    in0=ctxt[pr // 2][rs, pr % 2, hh * 64:hh * 64 + 64],
                    scalar1=rsk[rs, pr:pr + 1], scalar2=SCALE,
                    op0=Al.mult, op1=Al.mult)
        if debug:
            nc.sync.dma_start(out=dbg["d_dw"], in_=dw)
            nc.sync.dma_start(out=dbg["d_expq"], in_=expq)
            nc.sync.dma_start(out=dbg["d_rsk"], in_=rsk)
            nc.sync.dma_start(out=dbg["d_ctxn"], in_=ctxn)

        ctxA.close()  # release pa + phC + phS
        if debug:
            nc.sync.dma_start(out=dbg["d_rb"], in_=rb)

        # ---- phase B2: eqn, att, gelu, out projection ----------------------
        eqp = ctx.enter_context(tc.tile_pool(name="eqp", bufs=12))
        attp = ctx.enter_context(
            tc.tile_pool(name="attp", bufs=3, space="PSUM"))
        outp = ctx.enter_context(
            tc.tile_pool(name="outp", bufs=2, space="PSUM"))
        osb = ctx.enter_context(tc.tile_pool(name="osb", bufs=3))

        def emit_att(c):
            csl = slice(c * 512, (c + 1) * 512)
            for ot in range(4):
                eqn = eqp.tile([128, 512], bf16, tag="eqn")
                nc.vector.tensor_mul(eqn, expq[:, ot, csl], rb[:, ot, csl])
                att = attp.tile([128, 512], f32, tag="att")
                nc.tensor.matmul(att, ctxn[:, ot], eqn, start=True, stop=True)
                nc.scalar.activation(expq[:, ot, csl], att, Act.Gelu)

        def emit_out(c):
            csl = slice(c * 512, (c + 1) * 512)
            for ct in range(2):
                op = outp.tile([128, 512], f32, tag="op")
                for ot in range(4):
                    nc.tensor.matmul(
                        op, wout[:, ot, ct * 128:(ct + 1) * 128],
                        expq[:, ot, csl], start=(ot == 0), stop=(ot == 3))
                ot_sb = osb.tile([128, 512], f32, tag="osb")
                nc.vector.tensor_scalar_add(ot_sb, op, bout2[:, ct:ct + 1])
                nc.sync.dma_start(out=out_d[ct, :, csl], in_=ot_sb)

        emit_att(0)
        for c in range(1, 8):
            emit_att(c)
            emit_out(c - 1)
        emit_out(7)

    nc.compile()
    return nc


def _prep_inputs(fmap, Wq, Wdw, Wkv, Wout, bout):
    bf16 = ml_dtypes.bfloat16
    f32 = np.float32

    def ctile(a):  # [256, X] -> [128, 2, X]
        return np.ascontiguousarray(
            a.reshape(2, 128, *a.shape[1:]).transpose(1, 0, *range(2, a.ndim + 1)))

    shared = {
        "wq": ctile(Wq.T.astype(bf16)),
        "wkv": ctile(Wkv.T.astype(bf16)),
        "wout": np.ascontiguousarray(
            Wout.T.astype(bf16).reshape(4, 128, C).transpose(1, 0, 2)),
        "wdw": np.ascontiguousarray(
            Wdw.reshape(C, 9).reshape(2, 128, 9).transpose(1, 0, 2).astype(f32)),
        "wdiag": np.ascontiguousarray(
            np.stack([np.diag(Wdw.reshape(C, 9)[:128, i]) for i in range(9)],
                     axis=1).astype(bf16)),
        "bout2": np.ascontiguousarray(bout.astype(f32).reshape(2, 128).T),
        "bdiag": np.kron(np.eye(2, dtype=f32), np.ones((64, 64), f32)).astype(bf16),
    }
    in_maps = []
    for b in range(B):
        fpa = np.pad(fmap[b], [(0, 0), (1, 1), (1, 1)]).astype(bf16)
        fpb = np.pad(fmap[b], [(0, 0), (1, 1), (2, 0)]).astype(bf16)
        m = dict(shared)
        m["fpa"] = ctile(fpa.reshape(C, NPAD))
        m["fpb"] = ctile(fpb.reshape(C, NPAD))
        in_maps.append(m)
    return in_maps


def kernel(fmap, Wq, Wdw, Wkv, Wout, bout, _trace=False, _tmpdir=None,
           _debug=False):
    from concourse.bass_utils import run_bass_kernel_spmd

    fmap, Wq, Wdw, Wkv, Wout, bout = (
        np.asarray(a, np.float32) for a in (fmap, Wq, Wdw, Wkv, Wout, bout))

    key = "nc_dbg" if _debug else "nc"
    if key not in _CACHE:
        _CACHE[key] = _build(debug=_debug)
    nc = _CACHE[key]

    in_maps = _prep_inputs(fmap, Wq, Wdw, Wkv, Wout, bout)
    res = run_bass_kernel_spmd(
        nc, in_maps, core_ids=list(range(B)), trace=_trace, tmpdir=_tmpdir)
    _CACHE["last_result"] = res
    out = np.stack([r["out"] for r in res.results])        # [B, 2, 128, N]
    return out.reshape(B, C, H, W).astype(np.float32)
